# revision 18
# baseline (speedup 1.0000x reference)
"""ANI-style per-species MLP (384->160->128->96->1, CELU) over [B=128, A=512]
atoms with species routing, atom-summed to [B]. 8-core SPMD Trainium2 kernel.

Sharding: atom-parallel. Atoms are grouped by species and dealt round-robin to
the 8 cores so every core sees the same per-species group sizes (padded with
zero-AEV dummy atoms whose contribution is subtracted on the host). Each core
streams its [384, slots*128] transposed AEV block, runs the 4 layers with
per-species weights stationary on the PE (float32r, fp32 PSUM accumulate),
and emits a per-molecule partial sum; the host adds the 8 partials.

CELU is computed as celu(z) = relu(z) + min(alpha*e^(z/alpha) - alpha, 0)
in 3 engine passes (ACT exp; a clamp on DVE 2x or as an ACT relu; a DVE
scalar_tensor_tensor combine); bias constants fold into the exp/relu
per-partition operands and the layer biases fold forward on the host.
"""

import os
import sys

import numpy as np

try:
    import concourse  # noqa: F401
except ImportError:
    sys.path.insert(0, "/opt/trn_rl_repo")

N_CORES = 8
B, A, FEAT = 128, 512, 384
N_SPECIES = 4
H0, H1, H2 = 160, 128, 96
ALPHA = 0.1
LNA = float(np.log(ALPHA))

WPS = 848  # weight-pack columns per species
CPS = 8    # constant-pack columns per species

TRACE = bool(int(os.environ.get("BASSNN_TRACE", "0")))
LAST = {}

_progs = {}


def _maybe_register_ntff_hook():
    try:
        import types

        import antenv
        from antenv import axon_hooks  # noqa: F401
        return
    except ImportError:
        pass
    try:
        import types

        import antenv
        from trn_agent_boot.trn_boot import _ntff_profile_via_ctypes

        mod = types.ModuleType("antenv.axon_hooks")
        holder = [None]
        mod.set_axon_ntff_profile_hook = lambda h: holder.__setitem__(0, h)
        mod.get_axon_ntff_profile_hook = lambda: holder[0]
        sys.modules["antenv.axon_hooks"] = mod
        antenv.axon_hooks = mod
        mod.set_axon_ntff_profile_hook(
            _ntff_profile_via_ctypes("/opt/axon/libaxon_pjrt.so")
        )
    except Exception:
        pass


def _tiles_for_groups(G):
    """Per-species padded group sizes -> list of (species, slot0, n_atoms)."""
    tiles = []
    slot0 = 0
    for s, g in enumerate(G):
        a = 0
        while a < g:
            na = 4 if g - a >= 4 else g - a
            tiles.append((s, slot0 + a, na))
            a += na
        slot0 += g
    return tiles


def _build_program(G, S):
    import concourse.bass as bass
    import concourse.tile as tile
    from concourse import bacc, mybir

    F32 = mybir.dt.float32
    F32R = mybir.dt.float32r
    EXP = mybir.ActivationFunctionType.Exp
    MIN = mybir.AluOpType.min
    MAX = mybir.AluOpType.max
    ADD = mybir.AluOpType.add

    RELU = mybir.ActivationFunctionType.Relu
    SUB = mybir.AluOpType.subtract

    tiles = _tiles_for_groups(G)
    ntiles = len(tiles)
    batches = [list(range(b, min(b + 4, ntiles))) for b in range(0, ntiles, 4)]
    nbatches = len(batches)
    WB0 = WPS * N_SPECIES
    CB0 = CPS * N_SPECIES + 1

    nc = bacc.Bacc("TRN2", target_bir_lowering=False, debug=False,
                   num_devices=N_CORES)
    xt = nc.dram_tensor("xt", [128, 3, S, 128], F32R, kind="ExternalInput").ap()
    wp = nc.dram_tensor("wp", [128, WPS * N_SPECIES + 128 * nbatches], F32R,
                        kind="ExternalInput").ap()
    cp = nc.dram_tensor("cp", [128, CPS * N_SPECIES + 1 + 2 * nbatches], F32,
                        kind="ExternalInput").ap()
    yo = nc.dram_tensor("yo", [1, 128], F32, kind="ExternalOutput").ap()

    with tile.TileContext(nc) as tc:
        with (
            tc.tile_pool(name="wpool", bufs=1) as wpool,
            tc.tile_pool(name="cpool", bufs=1) as cpool,
            tc.tile_pool(name="xpool", bufs=8) as xpool,
            tc.tile_pool(name="epool", bufs=4) as epool,
            tc.tile_pool(name="ypool", bufs=4) as ypool,
            tc.tile_pool(name="y0apool", bufs=6) as y0apool,
            tc.tile_pool(name="opool", bufs=1) as opool,
            tc.tile_pool(name="pp0a", bufs=2, space="PSUM") as pp0a,
            tc.tile_pool(name="pp0b", bufs=1, space="PSUM") as pp0b,
            tc.tile_pool(name="pp1", bufs=2, space="PSUM") as pp1,
            tc.tile_pool(name="pp2", bufs=2, space="PSUM") as pp2,
            tc.tile_pool(name="pp3", bufs=1, space="PSUM") as pp3,
        ):
            w = wpool.tile([128, WPS * N_SPECIES + 128 * nbatches], F32R)
            nc.sync.dma_start(w[:], wp[:])
            c = cpool.tile([128, CPS * N_SPECIES + 1 + 2 * nbatches], F32)
            nc.sync.dma_start(c[:], cp[:])

            def wcol(s, off, n):
                return w[:, s * WPS + off: s * WPS + off + n]

            def ccol(s, k, parts):
                return c[0:parts, s * CPS + k: s * CPS + k + 1]

            p3 = pp3.tile([1, 512], F32)

            # celu(z) - c from psum P (z = P + c), as 3 passes:
            #   E = exp(10P + 10c + ln a)          [ACT]
            #   t' = min(E - a, 0)  in-place       [DVE 2x or ACT relu-form]
            #   y = (P max (-c)) + t'              [DVE stt]
            clamp_flip = [0]

            def celu_unit(y_ap, p_ap, e_tile, ebias, mbias):
                nc.scalar.activation(e_tile, p_ap, EXP, bias=ebias, scale=10.0)
                clamp_flip[0] = (clamp_flip[0] + 1) % 3
                if clamp_flip[0] != 0:
                    nc.vector.tensor_scalar(e_tile, e_tile, ALPHA, 0.0, SUB, MIN)
                    nc.vector.scalar_tensor_tensor(y_ap, p_ap, mbias, e_tile,
                                                   MAX, ADD)
                else:
                    acol = c[0:e_tile.shape[0],
                             CPS * N_SPECIES: CPS * N_SPECIES + 1]
                    nc.scalar.activation(e_tile, e_tile, RELU,
                                         bias=acol, scale=-1.0)
                    nc.vector.scalar_tensor_tensor(y_ap, p_ap, mbias, e_tile,
                                                   MAX, SUB)

            xts_of = {}
            xoff_of = {}
            for t0 in range(0, ntiles, 2):
                pair = [t0] + ([t0 + 1] if t0 + 1 < ntiles else [])
                span = sum(tiles[t][2] for t in pair)
                a0 = tiles[t0][1]
                xts = xpool.tile([128, 3 * 1024], F32R)
                off = 0
                for t in pair:
                    xts_of[t] = xts
                    xoff_of[t] = (off, span)
                    off += tiles[t][2]
                nc.sync.dma_start(
                    xts[:, 0: 3 * span * 128].rearrange(
                        "p (f a m) -> p f a m", f=3, a=span, m=128),
                    xt[:, :, a0: a0 + span, :],
                )

            for bi, batch in enumerate(batches):
                y0as = {}
                y0bs = {}
                for j, ti in enumerate(batch):
                    s, a0, na = tiles[ti]
                    N = na * 128
                    xts = xts_of[ti]
                    toff, tspan = xoff_of[ti]

                    p0a = pp0a.tile([128, 512], F32)
                    p0b = pp0b.tile([32, 512], F32)
                    for fc in range(3):
                        base = (fc * tspan + toff) * 128
                        rhs = xts[:, base: base + N]
                        nc.tensor.matmul(p0a[:, 0:N], wcol(s, fc * 160, 128),
                                         rhs, start=(fc == 0), stop=(fc == 2))
                        nc.tensor.matmul(p0b[:, 0:N],
                                         wcol(s, fc * 160 + 128, 32),
                                         rhs, start=(fc == 0), stop=(fc == 2))

                    ea = epool.tile([128, 512], F32)
                    y0a = y0apool.tile([128, 512], F32R)
                    celu_unit(y0a[:, 0:N], p0a[:, 0:N], ea[:, 0:N],
                              ccol(s, 0, 128), ccol(s, 2, 128))
                    y0as[ti] = y0a
                    eb = epool.tile([32, 512], F32)
                    y0b = ypool.tile([32, 512], F32R)
                    celu_unit(y0b[:, 0:N], p0b[:, 0:N], eb[:, 0:N],
                              ccol(s, 1, 32), ccol(s, 3, 32))
                    y0bs[ti] = y0b

                for j, ti in enumerate(batch):
                    s, a0, na = tiles[ti]
                    N = na * 128
                    y0a = y0as[ti]
                    y0b = y0bs[ti]
                    p1 = pp1.tile([128, 512], F32)
                    nc.tensor.matmul(p1[:, 0:N], wcol(s, 480, 128),
                                     y0a[:, 0:N], start=True, stop=False)
                    nc.tensor.matmul(p1[:, 0:N], wcol(s, 608, 128)[0:32, :],
                                     y0b[:, 0:N], start=False, stop=True)
                    e1 = epool.tile([128, 512], F32)
                    y1 = ypool.tile([128, 512], F32R)
                    celu_unit(y1[:, 0:N], p1[:, 0:N], e1[:, 0:N],
                              ccol(s, 4, 128), ccol(s, 5, 128))

                    p2 = pp2.tile([96, 512], F32)
                    nc.tensor.matmul(p2[:, 0:N], wcol(s, 736, 96), y1[:, 0:N],
                                     start=True, stop=True)
                    e2 = epool.tile([96, 512], F32)
                    y2 = ypool.tile([96, 512], F32R)
                    celu_unit(y2[:, 0:N], p2[:, 0:N], e2[:, 0:N],
                              ccol(s, 6, 96), ccol(s, 7, 96))

                    nc.tensor.matmul(p3[0:1, 0:N], wcol(s, 832, 1)[0:96, :],
                                     y2[:, 0:N], start=(ti == 0),
                                     stop=(ti == ntiles - 1),
                                     skip_group_check=True)

            t3 = opool.tile([1, 512], F32)
            nc.scalar.copy(t3[:], p3[:])
            f01 = opool.tile([1, 128], F32)
            nc.vector.tensor_add(f01[:], t3[0:1, 0:128], t3[0:1, 128:256])
            f23 = opool.tile([1, 128], F32)
            nc.vector.tensor_add(f23[:], t3[0:1, 256:384], t3[0:1, 384:512])
            fo = opool.tile([1, 128], F32)
            nc.vector.tensor_add(fo[:], f01[:], f23[:])
            nc.sync.dma_start(yo[:], fo[:])

    nc.compile()
    return nc


def _celu64(z):
    return np.where(z > 0, z, ALPHA * np.expm1(z / ALPHA))


def kernel(fullaev, species, W0, b0, W1, b1, W2, b2, W3, b3):
    from concourse import bass_utils

    fullaev = np.ascontiguousarray(np.asarray(fullaev, dtype=np.float32))
    species = np.asarray(species, dtype=np.int32)
    Ws = [np.asarray(w, dtype=np.float32) for w in (W0, W1, W2, W3)]
    bs = [np.asarray(b, dtype=np.float32) for b in (b0, b1, b2, b3)]

    # --- species grouping: per-core slot assignment ---------------------
    ids = [np.where(species == s)[0] for s in range(N_SPECIES)]
    n = [len(i) for i in ids]
    G = []
    for s in range(N_SPECIES):
        g = -(-n[s] // N_CORES) if n[s] else 0
        g += g % 2
        G.append(g)
    S = sum(G)
    key = tuple(G)
    if key not in _progs:
        _progs[key] = _build_program(G, S)
    nc = _progs[key]

    # --- fold constants (float64) ---------------------------------------
    corr = np.zeros(N_SPECIES)
    Kdum = np.zeros(N_SPECIES)
    tiles = _tiles_for_groups(G)
    batches = [list(range(b, min(b + 4, len(tiles)))) for b in range(0, len(tiles), 4)]
    nbatches = len(batches)
    CB0 = CPS * N_SPECIES + 1
    WB0 = WPS * N_SPECIES
    cpack = np.zeros((128, CB0 + 2 * nbatches), np.float32)
    cpack[:, CB0 - 1] = ALPHA
    wpack = np.zeros((128, WB0 + 128 * nbatches), np.float32)
    for bi, batch in enumerate(batches):
        for j, ti in enumerate(batch):
            s = tiles[ti][0]
            b0s = bs[0][s].astype(np.float64)
            cpack[32 * j: 32 * j + 32, CB0 + 2 * bi] = (
                10.0 * b0s[128:] + LNA)
            cpack[32 * j: 32 * j + 32, CB0 + 2 * bi + 1] = -b0s[128:]
            wpack[32 * j: 32 * j + 32, WB0 + 128 * bi: WB0 + 128 * (bi + 1)] = (
                Ws[1][s][:, 128:].T)
    for s in range(N_SPECIES):
        w0, w1, w2, w3 = (w[s].astype(np.float64) for w in Ws)
        bb0, bb1, bb2, bb3 = (b[s].astype(np.float64) for b in bs)
        c1 = bb1 + w1 @ bb0
        c2 = bb2 + w2 @ c1
        corr[s] = bb3[0] + w3[0] @ c2
        y = _celu64(bb0)
        y = _celu64(w1 @ y + bb1)
        y = _celu64(w2 @ y + bb2)
        Kdum[s] = w3[0] @ y + bb3[0]

        cb = s * CPS
        cpack[:, cb + 0] = 10.0 * bb0[:128] + LNA
        cpack[:32, cb + 1] = 10.0 * bb0[128:] + LNA
        cpack[:, cb + 2] = -bb0[:128]
        cpack[:32, cb + 3] = -bb0[128:]
        cpack[:, cb + 4] = 10.0 * c1 + LNA
        cpack[:, cb + 5] = -c1
        cpack[:96, cb + 6] = 10.0 * c2 + LNA
        cpack[:96, cb + 7] = -c2

        wb = s * WPS
        for fc in range(3):
            blk = Ws[0][s][:, fc * 128:(fc + 1) * 128].T  # [128in, 160out]
            wpack[:, wb + fc * 160: wb + fc * 160 + 128] = blk[:, :128]
            wpack[:, wb + fc * 160 + 128: wb + (fc + 1) * 160] = blk[:, 128:]
        wpack[:, wb + 480: wb + 608] = Ws[1][s][:, :128].T
        wpack[:32, wb + 608: wb + 736] = Ws[1][s][:, 128:].T
        wpack[:, wb + 736: wb + 832] = Ws[2][s].T
        wpack[:96, wb + 832] = Ws[3][s][0, :]

    # --- per-core transposed, species-sorted AEV blocks -----------------
    in_maps = []
    dummy_counts = np.zeros((N_CORES, N_SPECIES), np.int64)
    for cid in range(N_CORES):
        xtc = np.zeros((128, 3, S, 128), np.float32)
        slot0 = 0
        for s in range(N_SPECIES):
            mine = ids[s][cid::N_CORES]
            nr = len(mine)
            dummy_counts[cid, s] = G[s] - nr
            if nr:
                g = fullaev[:, mine, :]               # [128, nr, 384]
                t = g.transpose(2, 1, 0)              # [384, nr, 128]
                xtc[:, :, slot0: slot0 + nr, :] = (
                    t.reshape(3, 128, nr, 128).transpose(1, 0, 2, 3)
                )
            slot0 += G[s]
        in_maps.append({"xt": xtc, "wp": wpack, "cp": cpack})

    if TRACE:
        _maybe_register_ntff_hook()
    res = bass_utils.run_bass_kernel_spmd(
        nc, in_maps, core_ids=list(range(N_CORES)), trace=TRACE
    )
    LAST["exec_time_ns"] = res.exec_time_ns
    LAST["trace"] = res.instructions_and_trace[1] if res.instructions_and_trace else None

    total_corr = 0.0
    for s in range(N_SPECIES):
        total_corr += N_CORES * G[s] * corr[s] - dummy_counts[:, s].sum() * Kdum[s]
    out = np.zeros(128, np.float64)
    for cid in range(N_CORES):
        out += res.results[cid]["yo"][0].astype(np.float64)
    out += total_corr
    return out.astype(np.float32)
